# revision 76
# baseline (speedup 1.0000x reference)
"""AttentionPool kernel for nn_AttentionPool_7215545057869 (Bass/Tile, TRN2).

Contract: kernel(**inputs) takes FULL (unsharded) inputs, returns the FULL
output [8, 128, 1024] float32.

Distribution: data-parallel over batch -- the 8 batch elements map 1:1 onto
the 8 NeuronCores (SPMD: same program, per-core input slices).

Per-core dataflow (one batch element, nq=128, nkv=4096, 16 heads x 64):
  phase 0: LayerNorm(q) -> Q = qn @ Wq -> per-head RMS-norm -> transpose,
           folding 64*gamma_q*gamma_k into the transposed Q scale.
  phase A: stream kv in 32 tiles of [128, 1024]: mask+cast fp16 ->
           PE-transpose -> KVp = kv @ Wkv (PSUM) -> RMS-norm K rows
           (free-dim reduce) -> PE-transpose K-hat into per-head K^T;
           V kept natural in SBUF (fp16).
  phase B: per head: dots = Q^T.T @ K^T (PSUM, N=512 chunks) ->
           E = exp(dots - 64) on ACT with accum_out row-sums
           (|dots| <= 64 by Cauchy-Schwarz so no max-pass needed) ->
           PE-transpose E -> attn@V accumulation -> divide by the
           softmax denominator on the small [128, 64] head output.
  phase C: transpose the head-concat output, project through Wout, DMA out.
"""

import sys

for _p in ("/opt/trn_rl_repo", "/opt/pypackages"):
    if _p not in sys.path:
        sys.path.insert(0, _p)

import numpy as np

HEADS = 16
D = 64
DIM = 1024
INNER = 1024
NQ = 128
NKV = 4096
NT = NKV // 128          # 32 kv tiles
KC = DIM // 128          # 8 contraction chunks
LN_EPS = 1e-5

_BUILT = None


def _build():
    import concourse.bass as bass
    import concourse.bacc as bacc
    import concourse.tile as tile
    import concourse.mybir as mybir
    from concourse.masks import make_identity
    from contextlib import ExitStack

    f32 = mybir.dt.float32
    f16 = mybir.dt.float16
    bf16 = mybir.dt.bfloat16
    AX = mybir.AxisListType
    OP = mybir.AluOpType
    AF = mybir.ActivationFunctionType

    nc = bacc.Bacc("TRN2", target_bir_lowering=False, debug=False)

    q_d = nc.dram_tensor("q", [NQ, DIM], f32, kind="ExternalInput")
    # kv pre-masked and pre-cast to fp16 on host
    kv_d = nc.dram_tensor("kv16", [NKV, DIM], f16, kind="ExternalInput")
    gsc_d = nc.dram_tensor("gsc", [128, KC], f32, kind="ExternalInput")
    # weights pre-cast to fp16 on host (ln_w folded into wq there)
    wq_d = nc.dram_tensor("wq16", [DIM, INNER], f16, kind="ExternalInput")
    wkv_d = nc.dram_tensor("wkv16", [DIM, 2 * INNER], f16, kind="ExternalInput")
    wout_d = nc.dram_tensor("wout16", [INNER, DIM], f16, kind="ExternalInput")
    out_d = nc.dram_tensor("out", [NQ, DIM], f32, kind="ExternalOutput")

    def bcast_last(ap, n):
        # [P, G] -> [P, G, n] with stride-0 innermost (broadcast)
        return bass.AP(ap.tensor, ap.offset, [*ap.ap, [0, n]])

    with tile.TileContext(nc) as tc, ExitStack() as top:
        # ---------- persistent pools ----------
        consts = top.enter_context(tc.tile_pool(name="consts", bufs=1))
        ktp = top.enter_context(tc.tile_pool(name="ktp", bufs=1))
        vp = top.enter_context(tc.tile_pool(name="vp", bufs=1))

        eps_sb = consts.tile([128, 1], f32)
        nc.vector.memset(eps_sb, LN_EPS)
        neg64_sb = consts.tile([128, 1], f32)
        nc.vector.memset(neg64_sb, -64.0)

        gsc_sb = consts.tile([128, KC], f32)
        nc.sync.dma_start(gsc_sb, gsc_d[:])

        # K^T (normalized), per-head layout: [p, c, m]  inner = c*128+p
        kt_sb = ktp.tile([128, KC, NKV], f16)
        # V natural: [p, t, j]  row = t*128+p, j = h*64+d
        v_sb = vp.tile([128, NT, INNER], f16)

        # QT: [p, c, nq] -- transposed normalized+scaled Q
        qt_sb = consts.tile([128, KC, NQ], f16)
        # concat head outputs (already divided by softmax denom)
        outc_sb = consts.tile([128, INNER], f16)

        # p0a/wq pools created before phase-A pools (LIFO close order; also
        # keeps them out of the phase-A SBUF region so their DMAs don't wait
        # on phase-A readers), instructions emitted later
        ph0a = ExitStack()
        p0a = ph0a.enter_context(tc.tile_pool(name="p0a", bufs=1))
        wqp = ph0a.enter_context(tc.tile_pool(name="wqp", bufs=1))
        # first half of Wq (cols 0-511), loaded at program start
        wq_h0 = wqp.tile([128, KC, 512], f16, tag="wqh0")
        for c in range(KC):
            nc.scalar.dma_start(wq_h0[:, c, :], wq_d[c * 128:(c + 1) * 128, :512])

        # ---------- phase A pools (created early so weight/kv DMAs lead) ----
        phA = top.enter_context(ExitStack())
        pa = phA.enter_context(tc.tile_pool(name="pa", bufs=2))
        pamm = phA.enter_context(tc.tile_pool(name="pamm", bufs=2))
        wkv_pool = phA.enter_context(tc.tile_pool(name="wkv16p", bufs=1))
        kvps_p = phA.enter_context(tc.tile_pool(name="kvps", bufs=2, space="PSUM"))

        wkv_sb = wkv_pool.tile([128, KC, 2 * INNER], f16, tag="wkv16")
        for c in range(KC):
            nc.sync.dma_start(wkv_sb[:, c, :], wkv_d[c * 128:(c + 1) * 128, :])

        def start_load(t):
            kvm = pamm.tile([128, DIM], f16, tag="kvm")
            nc.sync.dma_start(kvm, kv_d[t * 128:(t + 1) * 128, :])
            return kvm

        def start_transpose(kvm):
            # transpose kv tile via DMA xbar (fp16)
            kvt = pamm.tile([128, KC, 128], f16, tag="kvt")
            nc.sync.dma_start_transpose(kvt, kvm)
            return kvt

        # pure-DMA input chain: loads lead transposes by one tile so the
        # transpose's input wait never blocks the next load's issue
        kvms = {0: start_load(0), 1: start_load(1)}
        kvts = {0: start_transpose(kvms.pop(0))}

        qnt = None

        def emit_ln_chain():
            # phase 0a: q LayerNorm -> qn^T, emitted mid-phase-A so the
            # serial chain overlaps the kv stream (ACT/DVE have slack there)
            nonlocal qnt
            q_sb = p0a.tile([NQ, DIM], f32, tag="q_in")
            nc.sync.dma_start(q_sb, q_d[:])
            ssum = p0a.tile([NQ, 1], f32, tag="ssum")
            nc.vector.tensor_reduce(ssum, q_sb, axis=AX.X, op=OP.add)
            mu = p0a.tile([NQ, 1], f32, tag="mu")
            nc.scalar.mul(mu, ssum, 1.0 / DIM)
            qc = p0a.tile([NQ, DIM], f16, tag="qc")
            nc.vector.tensor_scalar_sub(qc, q_sb, mu)
            sq = p0a.tile([NQ, DIM], f16, tag="qn")  # scratch; slot reused by qn
            var = p0a.tile([NQ, 1], f32, tag="var")
            nc.scalar.activation(sq, qc, AF.Square, accum_out=var)
            std = p0a.tile([NQ, 1], f32, tag="std")
            nc.scalar.activation(std, var, AF.Sqrt, bias=eps_sb, scale=1.0 / DIM)
            rstd = p0a.tile([NQ, 1], f32, tag="rstd")
            nc.vector.reciprocal(rstd, std)
            qn = p0a.tile([NQ, DIM], f16, tag="qn")
            nc.vector.tensor_scalar_mul(qn, qc, rstd)
            qnt = p0a.tile([128, KC, NQ], f16, tag="qnt")
            nc.sync.dma_start_transpose(qnt, qn)

        # ---------- phase A: KV stream ----------
        if True:  # (pools live in phA ExitStack, closed after the loop)
            for t in range(NT):
                kvt = kvts.pop(t)
                # KVp = kv @ Wkv  -> psum [128, 2048]
                kvp = kvps_p.tile([128, 2 * INNER], f32, tag="kvp")
                for nh in range(4):
                    for c in range(KC):
                        nc.tensor.matmul(
                            kvp[:, nh * 512:(nh + 1) * 512],
                            kvt[:, c, :],
                            wkv_sb[:, c, nh * 512:(nh + 1) * 512],
                            start=(c == 0), stop=(c == KC - 1))
                if t + 2 < NT:
                    kvms[t + 2] = start_load(t + 2)
                if t + 1 < NT:
                    kvts[t + 1] = start_transpose(kvms.pop(t + 1))
                if t == 20:
                    emit_ln_chain()

                # RMS norm of K rows (per head)
                kpv = kvp[:, :INNER].rearrange("p (h d) -> p h d", h=HEADS)
                ksq = pa.tile([128, HEADS, D], f16, tag="ksq")
                nc.scalar.activation(ksq, kpv, AF.Square)
                kss = pa.tile([128, HEADS], f32, tag="kss")
                nc.vector.tensor_reduce(kss, ksq, axis=AX.X, op=OP.add)
                knrm = pa.tile([128, HEADS], f32, tag="knrm")
                nc.scalar.sqrt(knrm, kss)
                # floor 1e-4 (not 1e-12) so 1/norm stays in f16 range for
                # masked (all-zero) rows; real row norms are ~8
                nc.vector.tensor_scalar_max(knrm, knrm, 1e-4)
                krs = pa.tile([128, HEADS], f32, tag="krs")
                nc.vector.reciprocal(krs, knrm)
                kn = pa.tile([128, HEADS, D], f16, tag="kn")
                for h in range(HEADS):
                    nc.vector.tensor_scalar_mul(kn[:, h, :], kpv[:, h, :],
                                                krs[:, h:h + 1])

                # transpose K-hat into KT via DMA xbar
                nc.sync.dma_start_transpose(
                    kt_sb[:, :, t * 128:(t + 1) * 128],
                    kn.rearrange("p h d -> p (h d)"))

                # V natural, fp16
                nc.scalar.copy(v_sb[:, t, :], kvp[:, INNER:])

        phA.close()  # release phase-A SBUF/PSUM pools before phase B

        # ---------- phase 0b: Q projection ----------
        with ExitStack() as ph0:
            p0 = ph0.enter_context(tc.tile_pool(name="p0", bufs=1))
            p0ps = ph0.enter_context(tc.tile_pool(name="p0ps", bufs=2, space="PSUM"))

            # wq on the scalar HWDGE queue: the sync sequencer is still
            # draining phase-A transposes at this point
            # Q = qn @ Wq  (psum [128, 512] x2); Wq half 0 was prefetched at
            # program start, half 1 streams through its own slot here
            wq_h1 = wqp.tile([128, KC, 512], f16, tag="wqh1")
            for c in range(KC):
                nc.scalar.dma_start(wq_h1[:, c, :],
                                    wq_d[c * 128:(c + 1) * 128, 512:])
            qproj = p0.tile([NQ, INNER], f32, tag="qproj")
            for nh, wqh in ((0, wq_h0), (1, wq_h1)):
                qps = p0ps.tile([NQ, 512], f32, tag="qps")
                for c in range(KC):
                    nc.tensor.matmul(
                        qps, qnt[:, c, :], wqh[:, c, :],
                        start=(c == 0), stop=(c == KC - 1))
                nc.vector.tensor_copy(qproj[:, nh * 512:(nh + 1) * 512], qps)

            # per-head RMS norm (no sqrt(d); folded into gsc)
            qsq = p0.tile([NQ, HEADS, D], f32, tag="qsq")
            nc.vector.tensor_mul(qsq, qproj.rearrange("n (h d) -> n h d", h=HEADS),
                                 qproj.rearrange("n (h d) -> n h d", h=HEADS))
            qss = p0.tile([NQ, HEADS], f32, tag="qss")
            nc.vector.tensor_reduce(qss, qsq, axis=AX.X, op=OP.add)
            qnrm = p0.tile([NQ, HEADS], f32, tag="qnrm")
            nc.scalar.sqrt(qnrm, qss)
            nc.vector.tensor_scalar_max(qnrm, qnrm, 1e-12)
            qrs = p0.tile([NQ, HEADS], f32, tag="qrs")
            nc.vector.reciprocal(qrs, qnrm)
            qhn = p0.tile([NQ, HEADS, D], f16, tag="qhn")
            qpv = qproj.rearrange("n (h d) -> n h d", h=HEADS)
            for h in range(HEADS):
                nc.vector.tensor_scalar_mul(qhn[:, h, :], qpv[:, h, :],
                                            qrs[:, h:h + 1])

            # transpose (DMA xbar) + gamma scale -> QT, in halves so phase B's
            # first heads can start before the full transpose completes
            qtr = p0.tile([128, KC, NQ], f16, tag="qtr")
            qhn2 = qhn.rearrange("n h d -> n (h d)")
            for half in range(2):
                h4 = KC // 2
                nc.scalar.dma_start_transpose(
                    qtr[:, half * h4:(half + 1) * h4, :],
                    qhn2[:, half * 512:(half + 1) * 512])
                for c in range(half * h4, (half + 1) * h4):
                    nc.vector.tensor_scalar_mul(qt_sb[:, c, :], qtr[:, c, :],
                                                gsc_sb[:, c:c + 1])

        ph0a.close()


        # ---------- phases B+C ----------
        with ExitStack() as phBC:
            wo_pool = phBC.enter_context(tc.tile_pool(name="wo16p", bufs=1))
            pbc = phBC.enter_context(tc.tile_pool(name="pbc", bufs=1))

            # prefetch Wout (fp16) during phase B
            wout_sb = wo_pool.tile([128, KC, DIM], f16, tag="wout16")
            nc.sync.dma_start(wout_sb, wout_d.rearrange("(c p) n -> p c n", p=128))

            # ---------- phase B: attention ----------
            with ExitStack() as phB:
                pbs = phB.enter_context(tc.tile_pool(name="pbs", bufs=2))
                dots_p = phB.enter_context(
                    tc.tile_pool(name="dots", bufs=3, space="PSUM"))
                outps_p = phB.enter_context(
                    tc.tile_pool(name="outps", bufs=2, space="PSUM"))

                def emit_head(h, prev):
                    # emits head h's QK^T/exp, finely interleaved with head
                    # h-1's attn@V so the PE stream stays dense
                    c = h // 2
                    pr = (h % 2) * 64
                    lhs_q = qt_sb[pr:pr + 64, c, :]
                    e_h = pbs.tile([NQ, NKV], bf16, tag="e")
                    dsum = pbs.tile([NQ, 4], f32, tag="dsum")
                    et = pbs.tile([128, NT, NQ], bf16, tag="et")
                    outh_prev = None
                    if prev is not None:
                        outh_prev = outps_p.tile([NQ, D], f32, tag="outh")
                    for cc in range(4):
                        dots = dots_p.tile([NQ, 1024], f32, tag="dots")
                        for half in range(2):
                            nc.tensor.matmul(
                                dots[:, half * 512:(half + 1) * 512], lhs_q,
                                kt_sb[pr:pr + 64, c,
                                      cc * 1024 + half * 512:
                                      cc * 1024 + (half + 1) * 512],
                                start=True, stop=True)
                        nc.scalar.activation(
                            e_h[:, cc * 1024:(cc + 1) * 1024], dots, AF.Exp,
                            bias=neg64_sb, scale=1.0, accum_out=dsum[:, cc:cc + 1])
                        if cc == 1:
                            nc.sync.dma_start_transpose(
                                et[:, :NT // 2, :], e_h[:, :NKV // 2])
                        elif cc == 3:
                            nc.sync.dma_start_transpose(
                                et[:, NT // 2:, :], e_h[:, NKV // 2:])
                        if prev is not None:
                            et_p, _, hp = prev
                            for k in range(cc * 8, cc * 8 + 8):
                                nc.tensor.matmul(
                                    outh_prev, et_p[:, k, :],
                                    v_sb[:, k, hp * 64:hp * 64 + 64],
                                    start=(k == 0), stop=(k == NT - 1))
                    if prev is not None:
                        _, rden_p, hp = prev
                        nc.vector.tensor_scalar_mul(
                            outc_sb[:, hp * 64:hp * 64 + 64], outh_prev, rden_p)
                    den = pbs.tile([NQ, 1], f32, tag="den")
                    nc.vector.tensor_reduce(den, dsum, axis=AX.X, op=OP.add)
                    rden = pbs.tile([NQ, 1], f32, tag="rden")
                    nc.vector.reciprocal(rden, den)
                    return et, rden, h

                outt = pbc.tile([128, KC, NQ], f16, tag="outt")
                prev = None
                for h in range(HEADS):
                    prev = emit_head(h, prev)
                    if h == 9:
                        # heads 0-7 retired (head 8's emission flushed 7's
                        # attn@V): transpose the first half of the output
                        nc.scalar.dma_start_transpose(
                            outt[:, :KC // 2, :], outc_sb[:, :INNER // 2])
                # flush the last head's attn@V
                et_p, rden_p, hp = prev
                outh_l = outps_p.tile([NQ, D], f32, tag="outh")
                for k in range(NT):
                    nc.tensor.matmul(
                        outh_l, et_p[:, k, :], v_sb[:, k, hp * 64:hp * 64 + 64],
                        start=(k == 0), stop=(k == NT - 1))
                nc.vector.tensor_scalar_mul(
                    outc_sb[:, hp * 64:hp * 64 + 64], outh_l, rden_p)
                nc.scalar.dma_start_transpose(
                    outt[:, KC // 2:, :], outc_sb[:, INNER // 2:])

            # ---------- phase C: output projection ----------
            pc_ps = phBC.enter_context(tc.tile_pool(name="pcps", bufs=2, space="PSUM"))
            fo_sb = pbc.tile([NQ, DIM], f32, tag="fo")
            for nh in range(2):
                fps = pc_ps.tile([NQ, 512], f32, tag="fps")
                for cch in range(KC):
                    nc.tensor.matmul(
                        fps, outt[:, cch, :], wout_sb[:, cch, nh * 512:(nh + 1) * 512],
                        start=(cch == 0), stop=(cch == KC - 1))
                nc.vector.tensor_copy(fo_sb[:, nh * 512:(nh + 1) * 512], fps)
            nc.sync.dma_start(out_d[:], fo_sb)

    nc.compile()
    return nc


def _get_built():
    global _BUILT
    if _BUILT is None:
        _BUILT = _build()
    return _BUILT


def _prep_in_maps(q, kv, mask, ln_w, gamma_q, gamma_k, Wq, Wkv, Wout):
    q = np.ascontiguousarray(np.asarray(q, dtype=np.float32))
    kv = np.ascontiguousarray(np.asarray(kv, dtype=np.float32))
    mask = np.asarray(mask).astype(np.float32)
    g = (64.0 * np.asarray(gamma_q, np.float32)[:, 0, :]
         * np.asarray(gamma_k, np.float32)[:, 0, :]).reshape(-1)
    gsc = np.ascontiguousarray(g.reshape(KC, 128).T.astype(np.float32))
    lnw = np.asarray(ln_w, np.float32)
    wq16 = np.ascontiguousarray(
        (lnw[:, None] * np.asarray(Wq, np.float32)).astype(np.float16))
    wkv16 = np.ascontiguousarray(np.asarray(Wkv, np.float32).astype(np.float16))
    wout16 = np.ascontiguousarray(np.asarray(Wout, np.float32).astype(np.float16))
    in_maps = []
    for b in range(q.shape[0]):
        kv16 = (kv[b] * mask[b][:, None]).astype(np.float16)
        in_maps.append({
            "q": q[b], "kv16": kv16,
            "gsc": gsc, "wq16": wq16, "wkv16": wkv16, "wout16": wout16,
        })
    return in_maps


def kernel(q, kv, mask, ln_w, gamma_q, gamma_k, Wq, Wkv, Wout):
    from concourse.bass_utils import run_bass_kernel_spmd

    nc = _get_built()
    in_maps = _prep_in_maps(q, kv, mask, ln_w, gamma_q, gamma_k, Wq, Wkv, Wout)
    res = run_bass_kernel_spmd(nc, in_maps, core_ids=list(range(len(in_maps))))
    out = np.stack([r["out"] for r in res.results]).astype(np.float32)
    return out


# revision 77
# speedup vs baseline: 1.3192x; 1.3192x over previous
"""AttentionPool kernel for nn_AttentionPool_7215545057869 (Bass/Tile, TRN2).

Contract: kernel(**inputs) takes FULL (unsharded) inputs, returns the FULL
output [8, 128, 1024] float32.

Distribution: data-parallel over batch -- the 8 batch elements map 1:1 onto
the 8 NeuronCores (SPMD: same program, per-core input slices).

Per-core dataflow (one batch element, nq=128, nkv=4096, 16 heads x 64):
  phase 0: LayerNorm(q) -> Q = qn @ Wq -> per-head RMS-norm -> transpose,
           folding 64*gamma_q*gamma_k into the transposed Q scale.
  phase A: stream kv in 32 tiles of [128, 1024]: mask+cast fp16 ->
           PE-transpose -> KVp = kv @ Wkv (PSUM) -> RMS-norm K rows
           (free-dim reduce) -> PE-transpose K-hat into per-head K^T;
           V kept natural in SBUF (fp16).
  phase B: per head: dots = Q^T.T @ K^T (PSUM, N=512 chunks) ->
           E = exp(dots - 64) on ACT with accum_out row-sums
           (|dots| <= 64 by Cauchy-Schwarz so no max-pass needed) ->
           PE-transpose E -> attn@V accumulation -> divide by the
           softmax denominator on the small [128, 64] head output.
  phase C: transpose the head-concat output, project through Wout, DMA out.
"""

import sys

for _p in ("/opt/trn_rl_repo", "/opt/pypackages"):
    if _p not in sys.path:
        sys.path.insert(0, _p)

import numpy as np

HEADS = 16
D = 64
DIM = 1024
INNER = 1024
NQ = 128
NKV = 4096
NT = NKV // 128          # 32 kv tiles
KC = DIM // 128          # 8 contraction chunks
LN_EPS = 1e-5

_BUILT = None


def _build():
    import concourse.bass as bass
    import concourse.bacc as bacc
    import concourse.tile as tile
    import concourse.mybir as mybir
    from concourse.masks import make_identity
    from contextlib import ExitStack

    f32 = mybir.dt.float32
    f16 = mybir.dt.float16
    bf16 = mybir.dt.bfloat16
    AX = mybir.AxisListType
    OP = mybir.AluOpType
    AF = mybir.ActivationFunctionType

    nc = bacc.Bacc("TRN2", target_bir_lowering=False, debug=False)

    q_d = nc.dram_tensor("q", [NQ, DIM], f32, kind="ExternalInput")
    # kv pre-masked and pre-cast to fp16 on host
    kv_d = nc.dram_tensor("kv16", [NKV, DIM], f16, kind="ExternalInput")
    gsc_d = nc.dram_tensor("gsc", [128, KC], f32, kind="ExternalInput")
    # weights pre-cast to fp16 on host (ln_w folded into wq there)
    wq_d = nc.dram_tensor("wq16", [DIM, INNER], f16, kind="ExternalInput")
    wkv_d = nc.dram_tensor("wkv16", [DIM, 2 * INNER], f16, kind="ExternalInput")
    wout_d = nc.dram_tensor("wout16", [INNER, DIM], f16, kind="ExternalInput")
    out_d = nc.dram_tensor("out", [NQ, DIM], f32, kind="ExternalOutput")

    def bcast_last(ap, n):
        # [P, G] -> [P, G, n] with stride-0 innermost (broadcast)
        return bass.AP(ap.tensor, ap.offset, [*ap.ap, [0, n]])

    with tile.TileContext(nc) as tc, ExitStack() as top:
        # ---------- persistent pools ----------
        consts = top.enter_context(tc.tile_pool(name="consts", bufs=1))
        ktp = top.enter_context(tc.tile_pool(name="ktp", bufs=1))
        vp = top.enter_context(tc.tile_pool(name="vp", bufs=1))

        eps_sb = consts.tile([128, 1], f32)
        nc.vector.memset(eps_sb, LN_EPS)
        neg64_sb = consts.tile([128, 1], f32)
        nc.vector.memset(neg64_sb, -64.0)

        gsc_sb = consts.tile([128, KC], f32)
        nc.sync.dma_start(gsc_sb, gsc_d[:])

        # K^T (normalized), per-head layout: [p, c, m]  inner = c*128+p
        kt_sb = ktp.tile([128, KC, NKV], f16)
        # V natural: [p, t, j]  row = t*128+p, j = h*64+d
        v_sb = vp.tile([128, NT, INNER], f16)

        # QT: [p, c, nq] -- transposed normalized+scaled Q
        qt_sb = consts.tile([128, KC, NQ], f16)
        # concat head outputs (already divided by softmax denom)
        outc_sb = consts.tile([128, INNER], f16)

        # p0a pool created before phase-A pools (LIFO close order), but its
        # instructions are emitted mid-phase-A
        ph0a = ExitStack()
        p0a = ph0a.enter_context(tc.tile_pool(name="p0a", bufs=1))

        # ---------- phase A pools (created early so weight/kv DMAs lead) ----
        phA = top.enter_context(ExitStack())
        pa = phA.enter_context(tc.tile_pool(name="pa", bufs=2))
        pamm = phA.enter_context(tc.tile_pool(name="pamm", bufs=3))
        wkv_pool = phA.enter_context(tc.tile_pool(name="wkv16p", bufs=1))
        kvps_p = phA.enter_context(tc.tile_pool(name="kvps", bufs=2, space="PSUM"))

        wkv_sb = wkv_pool.tile([128, KC, 2 * INNER], f16, tag="wkv16")
        for c in range(KC):
            nc.sync.dma_start(wkv_sb[:, c, :], wkv_d[c * 128:(c + 1) * 128, :])

        def start_load(t):
            kvm = pamm.tile([128, DIM], f16, tag="kvm")
            nc.sync.dma_start(kvm, kv_d[t * 128:(t + 1) * 128, :])
            return kvm

        def start_transpose(kvm):
            # transpose kv tile via DMA xbar (fp16)
            kvt = pamm.tile([128, KC, 128], f16, tag="kvt")
            nc.sync.dma_start_transpose(kvt, kvm)
            return kvt

        # pure-DMA input chain: loads lead transposes by one tile so the
        # transpose's input wait never blocks the next load's issue
        kvms = {0: start_load(0), 1: start_load(1)}
        kvts = {0: start_transpose(kvms.pop(0))}

        qnt = None

        def emit_ln_chain():
            # phase 0a: q LayerNorm -> qn^T, emitted mid-phase-A so the
            # serial chain overlaps the kv stream (ACT/DVE have slack there)
            nonlocal qnt
            q_sb = p0a.tile([NQ, DIM], f32, tag="q_in")
            nc.sync.dma_start(q_sb, q_d[:])
            ssum = p0a.tile([NQ, 1], f32, tag="ssum")
            nc.vector.tensor_reduce(ssum, q_sb, axis=AX.X, op=OP.add)
            mu = p0a.tile([NQ, 1], f32, tag="mu")
            nc.scalar.mul(mu, ssum, 1.0 / DIM)
            qc = p0a.tile([NQ, DIM], f32, tag="qc")
            nc.vector.tensor_scalar_sub(qc, q_sb, mu)
            sq = p0a.tile([NQ, DIM], f16, tag="sq")
            var = p0a.tile([NQ, 1], f32, tag="var")
            nc.scalar.activation(sq, qc, AF.Square, accum_out=var)
            std = p0a.tile([NQ, 1], f32, tag="std")
            nc.scalar.activation(std, var, AF.Sqrt, bias=eps_sb, scale=1.0 / DIM)
            rstd = p0a.tile([NQ, 1], f32, tag="rstd")
            nc.vector.reciprocal(rstd, std)
            qn = p0a.tile([NQ, DIM], f16, tag="qn")
            nc.vector.tensor_scalar_mul(qn, qc, rstd)
            qnt = p0a.tile([128, KC, NQ], f16, tag="qnt")
            nc.sync.dma_start_transpose(qnt, qn)

        # ---------- phase A: KV stream ----------
        if True:  # (pools live in phA ExitStack, closed after the loop)
            for t in range(NT):
                kvt = kvts.pop(t)
                # KVp = kv @ Wkv  -> psum [128, 2048]
                kvp = kvps_p.tile([128, 2 * INNER], f32, tag="kvp")
                for nh in range(4):
                    for c in range(KC):
                        nc.tensor.matmul(
                            kvp[:, nh * 512:(nh + 1) * 512],
                            kvt[:, c, :],
                            wkv_sb[:, c, nh * 512:(nh + 1) * 512],
                            start=(c == 0), stop=(c == KC - 1))
                if t + 2 < NT:
                    kvms[t + 2] = start_load(t + 2)
                if t + 1 < NT:
                    kvts[t + 1] = start_transpose(kvms.pop(t + 1))
                if t == 20:
                    emit_ln_chain()

                # RMS norm of K rows (per head)
                kpv = kvp[:, :INNER].rearrange("p (h d) -> p h d", h=HEADS)
                ksq = pa.tile([128, HEADS, D], f16, tag="ksq")
                nc.scalar.activation(ksq, kpv, AF.Square)
                kss = pa.tile([128, HEADS], f32, tag="kss")
                nc.vector.tensor_reduce(kss, ksq, axis=AX.X, op=OP.add)
                knrm = pa.tile([128, HEADS], f32, tag="knrm")
                nc.scalar.sqrt(knrm, kss)
                # floor 1e-4 (not 1e-12) so 1/norm stays in f16 range for
                # masked (all-zero) rows; real row norms are ~8
                nc.vector.tensor_scalar_max(knrm, knrm, 1e-4)
                krs = pa.tile([128, HEADS], f32, tag="krs")
                nc.vector.reciprocal(krs, knrm)
                # replicate krs [128,16] -> [128,16,64] with a stride-0-input
                # DVE copy, then a single elementwise multiply
                krep = pa.tile([128, HEADS, D], f16, tag="krep")
                nc.vector.tensor_copy(krep, bcast_last(krs, D))
                kn = pa.tile([128, HEADS, D], f16, tag="kn")
                nc.vector.tensor_tensor(kn, kpv, krep, op=OP.mult)

                # transpose K-hat into KT via DMA xbar
                nc.sync.dma_start_transpose(
                    kt_sb[:, :, t * 128:(t + 1) * 128],
                    kn.rearrange("p h d -> p (h d)"))

                # V natural, fp16
                nc.scalar.copy(v_sb[:, t, :], kvp[:, INNER:])

        phA.close()  # release phase-A SBUF/PSUM pools before phase B

        # ---------- phase 0b: Q projection ----------
        with ExitStack() as ph0:
            p0 = ph0.enter_context(tc.tile_pool(name="p0", bufs=1))
            p0ps = ph0.enter_context(tc.tile_pool(name="p0ps", bufs=2, space="PSUM"))

            # wq on the scalar HWDGE queue: the sync sequencer is still
            # draining phase-A transposes at this point
            wq_sb = p0.tile([128, KC, INNER], f16, tag="wq16")
            for c in range(KC):  # per-chunk DMAs spread across queues
                nc.scalar.dma_start(wq_sb[:, c, :], wq_d[c * 128:(c + 1) * 128, :])

            # Q = qn @ Wq  (psum [128, 512] x2)
            qproj = p0.tile([NQ, INNER], f32, tag="qproj")
            for nh in range(2):
                qps = p0ps.tile([NQ, 512], f32, tag="qps")
                for c in range(KC):
                    nc.tensor.matmul(
                        qps, qnt[:, c, :], wq_sb[:, c, nh * 512:(nh + 1) * 512],
                        start=(c == 0), stop=(c == KC - 1))
                nc.vector.tensor_copy(qproj[:, nh * 512:(nh + 1) * 512], qps)

            # per-head RMS norm (no sqrt(d); folded into gsc)
            qsq = p0.tile([NQ, HEADS, D], f32, tag="qsq")
            nc.vector.tensor_mul(qsq, qproj.rearrange("n (h d) -> n h d", h=HEADS),
                                 qproj.rearrange("n (h d) -> n h d", h=HEADS))
            qss = p0.tile([NQ, HEADS], f32, tag="qss")
            nc.vector.tensor_reduce(qss, qsq, axis=AX.X, op=OP.add)
            qnrm = p0.tile([NQ, HEADS], f32, tag="qnrm")
            nc.scalar.sqrt(qnrm, qss)
            nc.vector.tensor_scalar_max(qnrm, qnrm, 1e-12)
            qrs = p0.tile([NQ, HEADS], f32, tag="qrs")
            nc.vector.reciprocal(qrs, qnrm)
            qhn = p0.tile([NQ, HEADS, D], f16, tag="qhn")
            qpv = qproj.rearrange("n (h d) -> n h d", h=HEADS)
            for h in range(HEADS):
                nc.vector.tensor_scalar_mul(qhn[:, h, :], qpv[:, h, :],
                                            qrs[:, h:h + 1])

            # transpose (DMA xbar) + gamma scale -> QT, in halves so phase B's
            # first heads can start before the full transpose completes
            qtr = p0.tile([128, KC, NQ], f16, tag="qtr")
            qhn2 = qhn.rearrange("n h d -> n (h d)")
            for half in range(2):
                h4 = KC // 2
                nc.scalar.dma_start_transpose(
                    qtr[:, half * h4:(half + 1) * h4, :],
                    qhn2[:, half * 512:(half + 1) * 512])
                for c in range(half * h4, (half + 1) * h4):
                    nc.vector.tensor_scalar_mul(qt_sb[:, c, :], qtr[:, c, :],
                                                gsc_sb[:, c:c + 1])

        ph0a.close()


        # ---------- phases B+C ----------
        with ExitStack() as phBC:
            wo_pool = phBC.enter_context(tc.tile_pool(name="wo16p", bufs=1))
            pbc = phBC.enter_context(tc.tile_pool(name="pbc", bufs=1))

            # prefetch Wout (fp16) during phase B
            wout_sb = wo_pool.tile([128, KC, DIM], f16, tag="wout16")
            nc.sync.dma_start(wout_sb, wout_d.rearrange("(c p) n -> p c n", p=128))

            # ---------- phase B: attention ----------
            with ExitStack() as phB:
                pbs = phB.enter_context(tc.tile_pool(name="pbs", bufs=2))
                dots_p = phB.enter_context(
                    tc.tile_pool(name="dots", bufs=3, space="PSUM"))
                outps_p = phB.enter_context(
                    tc.tile_pool(name="outps", bufs=2, space="PSUM"))

                def emit_head(h, prev):
                    # emits head h's QK^T/exp, finely interleaved with head
                    # h-1's attn@V so the PE stream stays dense
                    c = h // 2
                    pr = (h % 2) * 64
                    lhs_q = qt_sb[pr:pr + 64, c, :]
                    e_h = pbs.tile([NQ, NKV], bf16, tag="e")
                    dsum = pbs.tile([NQ, 4], f32, tag="dsum")
                    et = pbs.tile([128, NT, NQ], bf16, tag="et")
                    outh_prev = None
                    if prev is not None:
                        outh_prev = outps_p.tile([NQ, D], f32, tag="outh")
                    for cc in range(4):
                        dots = dots_p.tile([NQ, 1024], f32, tag="dots")
                        for half in range(2):
                            nc.tensor.matmul(
                                dots[:, half * 512:(half + 1) * 512], lhs_q,
                                kt_sb[pr:pr + 64, c,
                                      cc * 1024 + half * 512:
                                      cc * 1024 + (half + 1) * 512],
                                start=True, stop=True)
                        nc.scalar.activation(
                            e_h[:, cc * 1024:(cc + 1) * 1024], dots, AF.Exp,
                            bias=neg64_sb, scale=1.0, accum_out=dsum[:, cc:cc + 1])
                        if cc == 1:
                            nc.sync.dma_start_transpose(
                                et[:, :NT // 2, :], e_h[:, :NKV // 2])
                        elif cc == 3:
                            nc.sync.dma_start_transpose(
                                et[:, NT // 2:, :], e_h[:, NKV // 2:])
                        if prev is not None:
                            et_p, _, hp = prev
                            for k in range(cc * 8, cc * 8 + 8):
                                nc.tensor.matmul(
                                    outh_prev, et_p[:, k, :],
                                    v_sb[:, k, hp * 64:hp * 64 + 64],
                                    start=(k == 0), stop=(k == NT - 1))
                    if prev is not None:
                        _, rden_p, hp = prev
                        nc.vector.tensor_scalar_mul(
                            outc_sb[:, hp * 64:hp * 64 + 64], outh_prev, rden_p)
                    den = pbs.tile([NQ, 1], f32, tag="den")
                    nc.vector.tensor_reduce(den, dsum, axis=AX.X, op=OP.add)
                    rden = pbs.tile([NQ, 1], f32, tag="rden")
                    nc.vector.reciprocal(rden, den)
                    return et, rden, h

                outt = pbc.tile([128, KC, NQ], f16, tag="outt")
                prev = None
                for h in range(HEADS):
                    prev = emit_head(h, prev)
                    if h == 9:
                        # heads 0-7 retired (head 8's emission flushed 7's
                        # attn@V): transpose the first half of the output
                        nc.scalar.dma_start_transpose(
                            outt[:, :KC // 2, :], outc_sb[:, :INNER // 2])
                # flush the last head's attn@V
                et_p, rden_p, hp = prev
                outh_l = outps_p.tile([NQ, D], f32, tag="outh")
                for k in range(NT):
                    nc.tensor.matmul(
                        outh_l, et_p[:, k, :], v_sb[:, k, hp * 64:hp * 64 + 64],
                        start=(k == 0), stop=(k == NT - 1))
                nc.vector.tensor_scalar_mul(
                    outc_sb[:, hp * 64:hp * 64 + 64], outh_l, rden_p)
                nc.scalar.dma_start_transpose(
                    outt[:, KC // 2:, :], outc_sb[:, INNER // 2:])

            # ---------- phase C: output projection ----------
            pc_ps = phBC.enter_context(tc.tile_pool(name="pcps", bufs=2, space="PSUM"))
            fo_sb = pbc.tile([NQ, DIM], f32, tag="fo")
            for nh in range(2):
                fps = pc_ps.tile([NQ, 512], f32, tag="fps")
                for cch in range(KC):
                    nc.tensor.matmul(
                        fps, outt[:, cch, :], wout_sb[:, cch, nh * 512:(nh + 1) * 512],
                        start=(cch == 0), stop=(cch == KC - 1))
                nc.vector.tensor_copy(fo_sb[:, nh * 512:(nh + 1) * 512], fps)
            nc.sync.dma_start(out_d[:], fo_sb)

    nc.compile()
    return nc


def _get_built():
    global _BUILT
    if _BUILT is None:
        _BUILT = _build()
    return _BUILT


def _prep_in_maps(q, kv, mask, ln_w, gamma_q, gamma_k, Wq, Wkv, Wout):
    q = np.ascontiguousarray(np.asarray(q, dtype=np.float32))
    kv = np.ascontiguousarray(np.asarray(kv, dtype=np.float32))
    mask = np.asarray(mask).astype(np.float32)
    g = (64.0 * np.asarray(gamma_q, np.float32)[:, 0, :]
         * np.asarray(gamma_k, np.float32)[:, 0, :]).reshape(-1)
    gsc = np.ascontiguousarray(g.reshape(KC, 128).T.astype(np.float32))
    lnw = np.asarray(ln_w, np.float32)
    wq16 = np.ascontiguousarray(
        (lnw[:, None] * np.asarray(Wq, np.float32)).astype(np.float16))
    wkv16 = np.ascontiguousarray(np.asarray(Wkv, np.float32).astype(np.float16))
    wout16 = np.ascontiguousarray(np.asarray(Wout, np.float32).astype(np.float16))
    in_maps = []
    for b in range(q.shape[0]):
        kv16 = (kv[b] * mask[b][:, None]).astype(np.float16)
        in_maps.append({
            "q": q[b], "kv16": kv16,
            "gsc": gsc, "wq16": wq16, "wkv16": wkv16, "wout16": wout16,
        })
    return in_maps


def kernel(q, kv, mask, ln_w, gamma_q, gamma_k, Wq, Wkv, Wout):
    from concourse.bass_utils import run_bass_kernel_spmd

    nc = _get_built()
    in_maps = _prep_in_maps(q, kv, mask, ln_w, gamma_q, gamma_k, Wq, Wkv, Wout)
    res = run_bass_kernel_spmd(nc, in_maps, core_ids=list(range(len(in_maps))))
    out = np.stack([r["out"] for r in res.results]).astype(np.float32)
    return out
